# revision 1
# baseline (speedup 1.0000x reference)
"""Tied-row (MSA) attention on 8 Trainium2 NeuronCores.

Reference computation (B=128, n=512, dim=256, h=8, dh=64,
r=tie_attn_dim=64, b=B//r=2):
    q = x @ Wq ; k,v = split(x @ Wkv)
    dots[b,h,i,j] = sum_{r,d} q[b,r,h,i,d] k[b,r,h,j,d] * scale
    attn = softmax_j(dots)
    out[b,r,h,i,d] = sum_j attn[b,h,i,j] v[b,r,h,j,d]
    y = out @ Wo + bo

Sharding: 8 cores = b(2) x head-pairs(4).  Each core owns one batch
element and 2 of the 8 heads and produces the partial
    y_part = out[:, :, own 2 heads, :] @ Wo[own 128 rows, :]
The host sums the 4 partials per b and adds bo (the head reduction of
the output projection commutes with the sum).

Per-core device kernel (shapes hardcoded):
  inputs : xT [64, 256, 512] f16   (x[b] transposed to [r, c, n])
           wq,wk,wv [256, 128] f16 (wq pre-scaled by dh^-.5 * r^-.5)
           wo [128, 256] f16
  output : y  [64, 512, 256] f32   (partial)

  Phase 1 + dots wave A fused (r-loop): qT_r/kT_r projections -> PSUM ->
          resident fp16 q_all/k_all [128=(2h x 64d), r, n]; dots for
          i-tiles 0,1 accumulate in 4 banks one iteration behind the
          copies (PSUM: 2 q + 2 k + 4 dots = 8 banks; the two heads'
          K=64 dots matmuls auto row-tile via base_partition 0/64 and
          run concurrently).  Wave A softmax inside this PSUM scope.
  Wave B: dots i-tiles 2,3 accumulate from resident q/k; attn tiles are
          transposed to attnT fp16 by single xbar DMA transposes
          (out[j, jc, i] = attn[i, jc*128+j]), overlapping the wave.
  Phase 3 (r-loop, 2-deep SW pipeline A=v, B=out, C=y): reload xT_r,
          v_r = xT_r.T @ wv, out_rT[hd, i] over j-chunks (lhsT = v f16,
          rhs = attnT f16; jc-outer/h-inner emission so the two heads'
          M=64 matmuls col-tile concurrently, skip_group_check for the
          interleaved PSUM groups), y_r[i, e] = out_rT.T @ wo, 8-row
          blocked DMA out on the ACT queue.

  Built with bacc.Bacc(): its compile() pass legalizes Tile's sync for
  this walrus (which caps sync waits per instruction); callers must
  finalize() the program before running (see _get_program).
"""

import os
import sys

for _p in ("/opt/trn_rl_repo", "/root/.axon_site/_ro/trn_rl_repo"):
    if os.path.isdir(_p) and _p not in sys.path:
        sys.path.insert(0, _p)

import numpy as np

R = 64          # tie dim (MSA rows per batch element)
RB = 8          # rows per DMA block
N = 512         # sequence length
C = 256         # model dim
HP = 128        # head-pair width: 2 heads x 64
E = 256         # output dim
NCORES = 8

_CACHE = {}


def build_program(phases=(1, 2, 3)):
    import concourse.bacc as bacc
    from concourse import mybir
    from concourse.tile import TileContext
    from contextlib import ExitStack

    f32 = mybir.dt.float32
    f16 = mybir.dt.float16

    # Bacc (not bass.Bass): its compile() pass legalizes sync for walrus --
    # moves matmul waits onto LDWEIGHTS and lowers multi-wait instructions
    # to event semaphores.  Raw Tile output violates walrus's per-struct
    # sync-wait limits.
    nc = bacc.Bacc()
    xT = nc.declare_dram_parameter("xT", [R, C, N], f16, isOutput=False)
    wq = nc.declare_dram_parameter("wq", [C, HP], f16, isOutput=False)
    wk = nc.declare_dram_parameter("wk", [C, HP], f16, isOutput=False)
    wv = nc.declare_dram_parameter("wv", [C, HP], f16, isOutput=False)
    wo = nc.declare_dram_parameter("wo", [HP, E], f16, isOutput=False)
    y = nc.declare_dram_parameter("y", [R, N, E], f32, isOutput=True)

    # xT block rb viewed as [p, r_in_block, c_chunk, n]
    xT_blk = xT.rearrange("(rb r) (cc p) n -> rb p r cc n", r=RB, p=128)
    # y block rb viewed as [p, r_in_block, i_tile, e]
    y_blk = y.rearrange("(rb r) (t p) e -> rb p r t e", r=RB, p=128)

    with TileContext(nc) as tc, ExitStack() as ctx:
        singles = ctx.enter_context(tc.tile_pool(name="singles", bufs=1))
        sm = ctx.enter_context(tc.tile_pool(name="sm", bufs=4))
        attnp = ctx.enter_context(tc.tile_pool(name="attnp", bufs=4))
        attntp = ctx.enter_context(tc.tile_pool(name="attntp", bufs=2))

        # weights: [256, X] -> sbuf [128, 2, X] (c-chunk on free axis)
        wq_sb = singles.tile([128, 2, HP], f16)
        wk_sb = singles.tile([128, 2, HP], f16)
        wv_sb = singles.tile([128, 2, HP], f16)
        wo_sb = singles.tile([128, E], f16)
        for cc in range(2):
            nc.gpsimd.dma_start(out=wq_sb[:, cc, :], in_=wq[cc * 128:(cc + 1) * 128, :])
            nc.gpsimd.dma_start(out=wk_sb[:, cc, :], in_=wk[cc * 128:(cc + 1) * 128, :])
            nc.gpsimd.dma_start(out=wv_sb[:, cc, :], in_=wv[cc * 128:(cc + 1) * 128, :])
        nc.gpsimd.dma_start(out=wo_sb, in_=wo[:, :])

        # attnT survives into phase 3: kernel-scoped pool
        attnT = [attntp.tile([128, 4, N], f16, tag="attnT", name=f"attnT_{h}")
                 for h in range(2)]

        def softmax(dots_hit, h, it):
            """dots PSUM tile -> normalized f16 attn SBUF tile.

            No max-subtraction: dots = q k^T with the 1/(sqrt(dh) sqrt(r))
            scale folded into Wq, so entries are ~N(0,1) and exp cannot
            overflow fp32/fp16.  This keeps ACT as the only dots reader
            (walrus allows at most 2 sync waits per instruction)."""
            ssum = sm.tile([128, 1], f32, tag="ssum", bufs=8)
            rinv = sm.tile([128, 1], f32, tag="rinv", bufs=8)
            attn = attnp.tile([128, N], f16, tag="attn", bufs=8,
                              name=f"attn_{h}_{it}")
            nc.scalar.activation(
                out=attn, in_=dots_hit,
                func=mybir.ActivationFunctionType.Exp,
                accum_out=ssum)
            nc.vector.reciprocal(rinv, ssum)
            nc.vector.tensor_scalar_mul(attn, attn, rinv)
            return attn

        def transpose_attn(ps_pool, attn, h, it):
            # one f16 xbar DMA transpose, SBUF -> SBUF: out[j, jc, i] =
            # attn[i, jc*128 + j]; no PE/PSUM involvement
            nc.sync.dma_start_transpose(
                out=attnT[h][:, :, it * 128:(it + 1) * 128], in_=attn)

        xpool = ctx.enter_context(tc.tile_pool(name="xpool", bufs=2))

        # resident fp16 qT/kT live only through phases 1-2
        with tc.tile_pool(name="resid", bufs=1) as resid:
            q_all = resid.tile([128, R, N], f16)
            k_all = resid.tile([128, R, N], f16)

            def dots_wave(dots_tiles, r, its):
                for it in its:
                    for h in range(2):
                        hs = slice(h * 64, (h + 1) * 64)
                        nc.tensor.matmul(
                            dots_tiles[h][it % 2],
                            lhsT=q_all[hs, r, it * 128:(it + 1) * 128],
                            rhs=k_all[hs, r, :],
                            start=(r == 0), stop=(r == R - 1))

            # -------- Phase 1 + dots wave A (i-tiles 0,1) fused --------
            attnA = {}
            with tc.tile_pool(name="ps1", space="PSUM", bufs=2) as ps1:
                dotsA = [[ps1.tile([128, N], f32, tag="dots", bufs=4,
                                   name=f"dotsA_{h}_{it}")
                          for it in range(2)] for h in range(2)]
                n_r = R if 1 in phases else 0
                for r in range(n_r + 1):
                    if r < n_r:
                        rb, ri = divmod(r, RB)
                        if ri == 0:
                            x_sb = xpool.tile([128, RB, 2, N], f16, tag="x",
                                              name=f"x1_{rb}")
                            nc.sync.dma_start(out=x_sb, in_=xT_blk[rb])
                        q_ps = ps1.tile([128, N], f32, tag="q")
                        k_ps = ps1.tile([128, N], f32, tag="k")
                        for cc in range(2):
                            nc.tensor.matmul(q_ps, lhsT=wq_sb[:, cc, :],
                                             rhs=x_sb[:, ri, cc, :],
                                             start=(cc == 0), stop=(cc == 1))
                        for cc in range(2):
                            nc.tensor.matmul(k_ps, lhsT=wk_sb[:, cc, :],
                                             rhs=x_sb[:, ri, cc, :],
                                             start=(cc == 0), stop=(cc == 1))
                        nc.vector.tensor_copy(q_all[:, r, :], q_ps)
                        nc.scalar.copy(k_all[:, r, :], k_ps)
                    if 0 <= r - 1 < n_r and 2 in phases:
                        dots_wave(dotsA, r - 1, (0, 1))
                # wave A softmax consumes the dots PSUM inside this scope
                for h in range(2 if 2 in phases else 0):
                    for it in range(2):
                        attnA[(h, it)] = softmax(dotsA[h][it], h, it)

            # -------- dots wave B + all transposes --------
            with tc.tile_pool(name="ps2", space="PSUM", bufs=2) as ps2:
                dotsB = [[ps2.tile([128, N], f32, tag="dots", bufs=4,
                                   name=f"dotsB_{h}_{it}")
                          for it in range(2)] for h in range(2)]
                for r in range(R if 2 in phases else 0):
                    dots_wave(dotsB, r, (2, 3))
                # wave A transposes overlap wave B's accumulation (PE is
                # in-order, but DVE copies and softmaxes interleave)
                for (h, it), attn in attnA.items():
                    transpose_attn(ps2, attn, h, it)
                for h in range(2 if 2 in phases else 0):
                    for it in (2, 3):
                        attn = softmax(dotsB[h][it % 2], h, it)
                        transpose_attn(ps2, attn, h, it)

        # ---------------- Phase 3: v, out, y (2-deep SW pipeline) ----------------
        with tc.tile_pool(name="ps3", space="PSUM", bufs=2) as ps3, \
             tc.tile_pool(name="vpool", bufs=4) as vpool, \
             tc.tile_pool(name="outp", bufs=4) as outp, \
             tc.tile_pool(name="ypool", bufs=2) as ypool:
            n_r = R if 3 in phases else 0
            v_sbs = {}
            out_sbs = {}
            y_sbs = {}

            def stage_a(r, x_sb, ri):
                v_ps = ps3.tile([128, 4, 128], f32, tag="v", name=f"v_ps_{r}")
                for jt in range(4):
                    for cc in range(2):
                        nc.tensor.matmul(
                            v_ps[:, jt, :],
                            lhsT=x_sb[:, ri, cc, jt * 128:(jt + 1) * 128],
                            rhs=wv_sb[:, cc, :],
                            start=(cc == 0), stop=(cc == 1))
                v_sb = vpool.tile([128, 4, 128], f16, tag="vsb", name=f"v_sb_{r}")
                nc.scalar.copy(v_sb, v_ps)
                v_sbs[r] = v_sb

            def stage_b(r):
                v_sb = v_sbs.pop(r)
                out_ps = ps3.tile([128, N], f32, tag="out", name=f"out_ps_{r}")
                # jc-outer / h-inner: adjacent matmuls hit different PE col
                # groups (out partitions 0-63 / 64-127) and run concurrently
                for jc in range(4):
                    for h in range(2):
                        hs = slice(h * 64, (h + 1) * 64)
                        nc.tensor.matmul(
                            out_ps[hs, :],
                            lhsT=v_sb[:, jc, hs],
                            rhs=attnT[h][:, jc, :],
                            start=(jc == 0), stop=(jc == 3),
                            skip_group_check=True)
                out_sb = outp.tile([128, N], f16, tag="outsb", name=f"out_sb_{r}")
                nc.vector.tensor_copy(out_sb, out_ps)
                out_sbs[r] = out_sb

            def stage_c(r):
                out_sb = out_sbs.pop(r)
                y_ps = ps3.tile([128, 4, E], f32, tag="y", name=f"y_ps_{r}")
                for it in range(4):
                    nc.tensor.matmul(
                        y_ps[:, it, :],
                        lhsT=out_sb[:, it * 128:(it + 1) * 128],
                        rhs=wo_sb,
                        start=True, stop=True)
                rb, ri = divmod(r, RB)
                if ri == 0:
                    y_sbs[rb] = ypool.tile([128, RB, 4, E], f32, tag="ysb",
                                           name=f"y_sb_{rb}")
                y_sb = y_sbs[rb]
                nc.vector.tensor_copy(y_sb[:, ri, 0:2, :], y_ps[:, 0:2, :])
                nc.scalar.copy(y_sb[:, ri, 2:4, :], y_ps[:, 2:4, :])
                if ri == RB - 1:
                    nc.scalar.dma_start(out=y_blk[rb], in_=y_sbs.pop(rb))

            x_tiles = {}
            for r in range(n_r + 2):
                if r < n_r:
                    rb, ri = divmod(r, RB)
                    if ri == 0:
                        x_tiles[rb] = xpool.tile([128, RB, 2, N], f16, tag="x",
                                                 name=f"x3_{rb}")
                        nc.sync.dma_start(out=x_tiles[rb], in_=xT_blk[rb])
                    stage_a(r, x_tiles[rb], ri)
                if 0 <= r - 1 < n_r:
                    stage_b(r - 1)
                if 0 <= r - 2 < n_r:
                    stage_c(r - 2)

    return nc


def _get_program():
    if "nc" not in _CACHE:
        nc = build_program()
        nc.finalize()
        _CACHE["nc"] = nc
    return _CACHE["nc"]


def make_in_maps(x, Wq, Wkv, Wo):
    """Host-side sharding: core = bi*4 + hpi."""
    scale = (64.0 ** -0.5) * (64.0 ** -0.5)
    x = np.asarray(x, np.float32)
    Wq = np.asarray(Wq, np.float32) * np.float32(scale)
    Wkv = np.asarray(Wkv, np.float32)
    Wo = np.asarray(Wo, np.float32)
    b = x.shape[0] // R
    xT = np.ascontiguousarray(
        x.reshape(b, R, N, C).transpose(0, 1, 3, 2)).astype(np.float16)
    in_maps = []
    for core in range(NCORES):
        bi, hpi = divmod(core, 4)
        cols = slice(hpi * HP, (hpi + 1) * HP)
        in_maps.append({
            "xT": xT[bi],
            "wq": np.ascontiguousarray(Wq[:, cols]).astype(np.float16),
            "wk": np.ascontiguousarray(Wkv[:, cols]).astype(np.float16),
            "wv": np.ascontiguousarray(
                Wkv[:, 512 + hpi * HP: 512 + (hpi + 1) * HP]).astype(np.float16),
            "wo": np.ascontiguousarray(Wo[cols, :]).astype(np.float16),
        })
    return in_maps


def combine_outputs(ys, bo):
    """ys: list of 8 [R, N, E] partials in core order; returns [B, n, dim]."""
    y0 = ys[0] + ys[1] + ys[2] + ys[3]
    y1 = ys[4] + ys[5] + ys[6] + ys[7]
    y = np.concatenate([y0, y1], axis=0).reshape(2 * R, N, E)
    return (y + np.asarray(bo, np.float32)).astype(np.float32)


def kernel(x, Wq, Wkv, Wo, bo, tie_attn_dim):
    assert int(tie_attn_dim) == R, f"hardcoded for tie_attn_dim={R}"
    from concourse.bass_utils import run_bass_kernel_spmd

    nc = _get_program()
    in_maps = make_in_maps(x, Wq, Wkv, Wo)
    res = run_bass_kernel_spmd(nc, in_maps, list(range(NCORES)))
    ys = [np.asarray(res.results[c]["y"], np.float32) for c in range(NCORES)]
    return combine_outputs(ys, bo)



# revision 52
# speedup vs baseline: 1.3600x; 1.3600x over previous
"""Tied-row (MSA) attention on 8 Trainium2 NeuronCores.

Reference computation (B=128, n=512, dim=256, h=8, dh=64,
r=tie_attn_dim=64, b=B//r=2):
    q = x @ Wq ; k,v = split(x @ Wkv)
    dots[b,h,i,j] = sum_{r,d} q[b,r,h,i,d] k[b,r,h,j,d] * scale
    attn = softmax_j(dots)
    out[b,r,h,i,d] = sum_j attn[b,h,i,j] v[b,r,h,j,d]
    y = out @ Wo + bo

Sharding: 8 cores = b(2) x head-pairs(4).  Each core owns one batch
element and 2 of the 8 heads and produces the partial
    y_part = out[:, :, own 2 heads, :] @ Wo[own 128 rows, :]
in f16; the host sums the 4 partials per b in f32 and adds bo.

Per-core device kernel (shapes hardcoded):
  inputs : xT [64, 256, 512] f16   (x[b] transposed to [r, c, n])
           wq,wk,wv [256, 128] f16 (wq pre-scaled by dh^-.5 * r^-.5)
           wo [128, 256] f16
  output : y  [64, 512, 256] f16   (partial)

All reductions feed the PE with full K=128 contraction chunks (the
cost dimension is the moving-free size only, so half-height K=64
matmuls waste PE):
  - dots contracts (r, d) in chunks of 128 by pairing consecutive MSA
    rows on the partition axis: qk2 [(r%2)*64+d, qk, h, rr, n] f16, so
    dots is 2h x 4i x 32rr matmuls of F=512 (vs 64 r-steps of K=64).
  - out[(r%2)*64+d, i] per (h, rr) uses v2 [j, jc, h, (r%2)*64+d] as
    stationary and attnT[h] [j, jc, i] as moving: 2h x 32rr x 4jc
    matmuls of F=512 with all 128 output partitions live.
  - y per r needs out in [hd, i] layout: 64-partition interleave
    copies out_ps[h][p*64:...] -> out_sb[h*64:...] recover K=128.

The (p, d) interleaves need PSUM->SBUF copies; q and k share one PSUM
tile and one SBUF destination so each r needs only TWO copies (one
per head, 1024 elems each), alternating DVE/ACT -- both engines stay
just under the PE's per-row-pair budget, so the PE never stalls and
keeps its p-state ramp.  GPSIMD cannot touch PSUM (BIR verifier), so
Pool only gets SBUF work (softmax scale) and the y writeback, which
is a casting SWDGE DMA straight from PSUM f32 to DRAM f16 (no engine
copy, no SBUF staging).

Phase 1 streams x once computing q/k per row; dots i-tiles 0,1
accumulate one r-pair behind the copies (PSUM: 2x2 qk + 4 dots).
Phase 2 runs dots i-tiles 2,3 from the resident qk2 while softmax
of wave one and the xbar DMA attn transposes overlap.  Phase 3
reloads x (v proj), then out/y in a SW pipeline
(PSUM: 2 v + 2 out + 2x2 y).

Built with bacc.Bacc(): its compile() pass legalizes Tile's sync for
walrus (which caps sync waits per instruction); callers must
finalize() the program before running (see _get_program).
"""

import os
import sys

for _p in ("/opt/trn_rl_repo", "/root/.axon_site/_ro/trn_rl_repo"):
    if os.path.isdir(_p) and _p not in sys.path:
        sys.path.insert(0, _p)

import numpy as np
from collections import deque

R = 64          # tie dim (MSA rows per batch element)
RR = 32         # r-pairs
RB = 8          # rows per qk staging/interleave block
XB = 4          # rows per x DMA block
RBY = 2         # rows per y DMA block
N = 512         # sequence length
C = 256         # model dim
HP = 128        # head-pair width: 2 heads x 64
E = 256         # output dim
NCORES = 8

_CACHE = {}


def build_program(phases=(1, 2, 3)):
    import concourse.bacc as bacc
    from concourse import mybir
    from concourse.tile import TileContext
    from contextlib import ExitStack

    f32 = mybir.dt.float32
    f16 = mybir.dt.float16

    nc = bacc.Bacc()
    xT = nc.declare_dram_parameter("xT", [R, C, N], f16, isOutput=False)
    wq = nc.declare_dram_parameter("wq", [C, HP], f16, isOutput=False)
    wk = nc.declare_dram_parameter("wk", [C, HP], f16, isOutput=False)
    wv = nc.declare_dram_parameter("wv", [C, HP], f16, isOutput=False)
    wo = nc.declare_dram_parameter("wo", [HP, E], f16, isOutput=False)
    y = nc.declare_dram_parameter("y", [R, N, E], f16, isOutput=True)

    xT_blk = xT.rearrange("(rb r) (cc p) n -> rb p r cc n", r=XB, p=128)
    y_blk = y.rearrange("(rb r) (t p) e -> rb p r t e", r=RBY, p=128)

    def copy_eng(e, out, in_):
        if e % 2 == 0:
            nc.vector.tensor_copy(out, in_)
        else:
            nc.scalar.copy(out, in_)

    with TileContext(nc) as tc, ExitStack() as ctx:
        singles = ctx.enter_context(tc.tile_pool(name="singles", bufs=1))
        sm = ctx.enter_context(tc.tile_pool(name="sm", bufs=4))
        attnp = ctx.enter_context(tc.tile_pool(name="attnp", bufs=4))
        attntp = ctx.enter_context(tc.tile_pool(name="attntp", bufs=2))
        xpool = ctx.enter_context(tc.tile_pool(name="xpool", bufs=3))
        qknp = ctx.enter_context(tc.tile_pool(name="qknp", bufs=4))

        # weights: [256, X] -> sbuf [128, 2, X] (c-chunk on free axis)
        wq_sb = singles.tile([128, 2, HP], f16)
        wk_sb = singles.tile([128, 2, HP], f16)
        wv_sb = singles.tile([128, 2, HP], f16)
        wo_sb = singles.tile([128, E], f16)

        # attnT survives into phase 3: kernel-scoped pool
        attnT = [attntp.tile([128, 4, N], f16, tag="attnT", name=f"attnT_{h}")
                 for h in range(2)]

        def softmax(dots_hit, h, it):
            """dots PSUM tile -> normalized f16 attn SBUF tile.

            No max-subtraction: dots = q k^T with the 1/(sqrt(dh) sqrt(r))
            scale folded into Wq, so entries are ~N(0,1) and exp cannot
            overflow fp32/fp16."""
            ssum = sm.tile([128, 1], f32, tag="ssum", bufs=8)
            rinv = sm.tile([128, 1], f32, tag="rinv", bufs=8)
            attn = attnp.tile([128, N], f16, tag="attn", bufs=4,
                              name=f"attn_{h}_{it}")
            nc.scalar.activation(
                out=attn, in_=dots_hit,
                func=mybir.ActivationFunctionType.Exp,
                accum_out=ssum)
            nc.vector.reciprocal(rinv, ssum)
            nc.gpsimd.tensor_scalar_mul(attn, attn, rinv)
            return attn

        def transpose_attn(attn, h, it):
            # one f16 xbar DMA transpose, SBUF -> SBUF: out[j, jc, i] =
            # attn[i, jc*128 + j]; no PE/PSUM involvement
            nc.sync.dma_start_transpose(
                out=attnT[h][:, :, it * 128:(it + 1) * 128], in_=attn)

        # resident interleaved q/k, one tile: [(r%2)*64+d, rr, qk, h, n] f16
        with tc.tile_pool(name="resid", bufs=1) as resid:
            qk2 = resid.tile([128, RR, 2, 2, N], f16)

            def dots_mm(tile, h, ic, rr):
                nc.tensor.matmul(
                    tile,
                    lhsT=qk2[:, rr, 0, h, ic * 128:(ic + 1) * 128],
                    rhs=qk2[:, rr, 1, h, :],
                    start=(rr == 0), stop=(rr == RR - 1))

            # -------- Phase 1 + dots i-tiles 0,1 fused --------
            attn01 = {}
            qk_nats = {}
            with tc.tile_pool(name="ps1", space="PSUM", bufs=2) as ps1:
                dots01 = [[ps1.tile([128, N], f32, tag="dots", bufs=4,
                                    name=f"dotsA_{h}_{ic}")
                           for ic in range(2)] for h in range(2)]
                n_r = R if 1 in phases else 0
                next_rr = 0
                pending_ints = deque()

                def emit_dots_up_to(limit, budget):
                    nonlocal next_rr
                    while next_rr < min(limit, RR) and budget > 0:
                        for h in range(2):
                            for ic in (0, 1):
                                dots_mm(dots01[h][ic], h, ic, next_rr)
                        next_rr += 1
                        budget -= 1

                for r in range(n_r + 16):
                    if r == 0:
                        # weights ride the Pool SWDGE queue (one DMA per
                        # tensor) in parallel with SP streaming the first x
                        # rows, so the first projection starts ~4us in
                        x_sb = xpool.tile([128, XB, 2, N], f16, tag="x",
                                          name="x1_0")
                        nc.sync.dma_start(out=x_sb[:, 0:2], in_=xT_blk[0, :, 0:2])
                        nc.sync.dma_start(
                            out=wq_sb, in_=wq.rearrange("(cc p) hp -> p cc hp", p=128))
                        nc.sync.dma_start(
                            out=wk_sb, in_=wk.rearrange("(cc p) hp -> p cc hp", p=128))
                        nc.sync.dma_start(out=x_sb[:, 2:XB], in_=xT_blk[0, :, 2:XB])
                        nc.sync.dma_start(
                            out=wv_sb, in_=wv.rearrange("(cc p) hp -> p cc hp", p=128))
                        nc.sync.dma_start(out=wo_sb, in_=wo[:, :])
                    if r < n_r:
                        rb, ri = divmod(r, XB)
                        if ri == 0 and rb > 0:
                            x_sb = xpool.tile([128, XB, 2, N], f16, tag="x",
                                              name=f"x1_{rb}")
                            nc.sync.dma_start(out=x_sb, in_=xT_blk[rb])
                        qk_ps = ps1.tile([128, 2, N], f32, tag="qk")
                        for cc in range(2):
                            nc.tensor.matmul(qk_ps[:, 0, :], lhsT=wq_sb[:, cc, :],
                                             rhs=x_sb[:, ri, cc, :],
                                             start=(cc == 0), stop=(cc == 1))
                        for cc in range(2):
                            nc.tensor.matmul(qk_ps[:, 1, :], lhsT=wk_sb[:, cc, :],
                                             rhs=x_sb[:, ri, cc, :],
                                             start=(cc == 0), stop=(cc == 1))
                        # one cheap full-partition f32->f16 staging copy per
                        # row (engines alternate); once 8 rows are staged,
                        # the 64-partition (h -> r%2) interleave runs as 4
                        # big f16 SBUF->SBUF SWDGE DMAs from the Pool queue
                        # (its own DGE queue: no head-of-line blocking
                        # behind SP's x loads).
                        blk, rb8 = divmod(r, RB)
                        if blk == n_r // RB - 1:
                            # last block: direct 64-partition interleave on
                            # the engines (they idle at the phase tail), so
                            # the final dots waves don't sit behind the
                            # ~8.5us staging-DMA chain
                            p, rr = r % 2, r // 2
                            ds = slice(p * 64, (p + 1) * 64)
                            nc.vector.tensor_copy(qk2[ds, rr, :, 0, :],
                                                  qk_ps[0:64, :, :])
                            nc.scalar.copy(qk2[ds, rr, :, 1, :],
                                           qk_ps[64:128, :, :])
                        else:
                            if rb8 == 0:
                                qk_nats[blk] = qknp.tile(
                                    [128, 2, 4, 2, N], f16, tag="qkn",
                                    bufs=2, name=f"qkn_{blk}")
                            copy_eng(r, qk_nats[blk][:, rb8 % 2, rb8 // 2, :, :],
                                     qk_ps)
                            if rb8 == RB - 1:
                                rrs = slice(blk * 4, (blk + 1) * 4)
                                for h in range(2):
                                    for pp in range(2):
                                        nc.gpsimd.dma_start(
                                            out=qk2[pp * 64:(pp + 1) * 64, rrs, :, h, :],
                                            in_=qk_nats[blk][h * 64:(h + 1) * 64, pp, :, :, :])
                    if 2 in phases and 1 in phases and r >= 11:
                        # last block's rows land fast (engine copies), the
                        # rest are gated on their staging DMAs
                        limit = min(4 * ((r - 11) // RB + 1),
                                    28 + max(0, (r - 57) // 2))
                        emit_dots_up_to(limit, 1 if r < n_r else RR)
                # wave-one softmax consumes the dots PSUM; wave-two tiles
                # rotate into the same tag's banks as the exps retire, so
                # the PE restarts on i-tiles 2,3 one exp-latency later
                # instead of waiting for a pool-scope boundary
                for h in range(2 if (2 in phases and 1 in phases) else 0):
                    for ic in (0, 1):
                        attn01[(h, ic)] = softmax(dots01[h][ic], h, ic)

                dots23 = [[ps1.tile([128, N], f32, tag="dots", bufs=4,
                                    name=f"dotsB_{h}_{ic}")
                           for ic in range(2)] for h in range(2)]
                for rr in range(RR if (2 in phases and 1 in phases) else 0):
                    for h in range(2):
                        for ic in (2, 3):
                            dots_mm(dots23[h][ic - 2], h, ic, rr)
                # wave-one transposes overlap wave two's accumulation
                for (h, ic), attn in attn01.items():
                    transpose_attn(attn, h, ic)
                for h in range(2 if (2 in phases and 1 in phases) else 0):
                    for ic in (2, 3):
                        attn = softmax(dots23[h][ic - 2], h, ic)
                        transpose_attn(attn, h, ic)

        # ---------------- Phase 3: v, out, y (SW pipeline) ---------
        with tc.tile_pool(name="ps3", space="PSUM", bufs=2) as ps3, \
             tc.tile_pool(name="vpool", bufs=3) as vpool, \
             tc.tile_pool(name="outp", bufs=4) as outp, \
             tc.tile_pool(name="ypool", bufs=2) as ypool:
            n_t = RR if 3 in phases else 0
            v2s = {}
            out_ps_s = {}
            y_sbs = {}

            def stage_a(r):
                rb, ri = divmod(r, XB)
                if ri == 0:
                    x_tiles[rb] = xpool.tile([128, XB, 2, N], f16,
                                             tag="x", name=f"x3_{rb}")
                    nc.sync.dma_start(out=x_tiles[rb], in_=xT_blk[rb])
                x_sb = x_tiles[rb]
                p, rr = r % 2, r // 2
                v_ps = ps3.tile([128, 4, 2, 64], f32, tag="v", name=f"v_ps_{r}")
                for jt in range(4):
                    for cc in range(2):
                        nc.tensor.matmul(
                            v_ps[:, jt],
                            lhsT=x_sb[:, ri, cc, jt * 128:(jt + 1) * 128],
                            rhs=wv_sb[:, cc, :],
                            start=(cc == 0), stop=(cc == 1))
                if p == 0:
                    v2s[rr] = vpool.tile([128, 4, 2, 128], f16, tag="v2",
                                         name=f"v2_{rr}")
                # both heads in one multi-dim-AP copy: src free (jt, h, d),
                # dst free (jc, h, d) with the parity offset on d
                copy_eng(r, v2s[rr][:, :, :, p * 64:(p + 1) * 64], v_ps)

            def stage_b(rr):
                o = [ps3.tile([128, N], f32, tag="out", bufs=2,
                              name=f"out_ps_{rr}_{h}") for h in range(2)]
                v2 = v2s.pop(rr)
                for jc in range(4):
                    for h in range(2):
                        nc.tensor.matmul(
                            o[h],
                            lhsT=v2[:, jc, h, :],
                            rhs=attnT[h][:, jc, :],
                            start=(jc == 0), stop=(jc == 3))
                out_ps_s[rr] = o

            def stage_c(r):
                p, rr = r % 2, r // 2
                o = out_ps_s[rr]
                out_sb = outp.tile([128, N], f16, tag="outsb",
                                   name=f"out_sb_{r}")
                for h in range(2):
                    copy_eng(r + h,
                             out_sb[h * 64:(h + 1) * 64, :],
                             o[h][p * 64:(p + 1) * 64, :])
                if p == 1:
                    out_ps_s.pop(rr)
                y_ps = ps3.tile([128, 4, E], f32, tag="y", bufs=2,
                                name=f"y_ps_{r}")
                for ic in range(4):
                    nc.tensor.matmul(
                        y_ps[:, ic, :],
                        lhsT=out_sb[:, ic * 128:(ic + 1) * 128],
                        rhs=wo_sb,
                        start=True, stop=True)
                yb, ryi = divmod(r, RBY)
                if ryi == 0:
                    y_sbs[yb] = ypool.tile([128, RBY, 4, E], f16, tag="ysb",
                                           name=f"y_sb_{yb}")
                copy_eng(r, y_sbs[yb][:, ryi, :, :], y_ps)
                if ryi == RBY - 1:
                    nc.gpsimd.dma_start(out=y_blk[yb], in_=y_sbs.pop(yb))

            # B first, then the next pair's A as PE filler while the
            # out copies land, then C whose y matmuls need those copies
            x_tiles = {}
            for t in range(n_t + 2):
                if 0 <= t - 1 < n_t:
                    stage_b(t - 1)
                if t < n_t:
                    stage_a(2 * t)
                    stage_a(2 * t + 1)
                if 0 <= t - 1 < n_t:
                    stage_c(2 * (t - 1))
                    stage_c(2 * (t - 1) + 1)

    return nc


def _get_program():
    if "nc" not in _CACHE:
        nc = build_program()
        nc.finalize()
        _CACHE["nc"] = nc
    return _CACHE["nc"]


def make_in_maps(x, Wq, Wkv, Wo):
    """Host-side sharding: core = bi*4 + hpi."""
    scale = (64.0 ** -0.5) * (64.0 ** -0.5)
    x = np.asarray(x, np.float32)
    Wq = np.asarray(Wq, np.float32) * np.float32(scale)
    Wkv = np.asarray(Wkv, np.float32)
    Wo = np.asarray(Wo, np.float32)
    b = x.shape[0] // R
    xT = np.ascontiguousarray(
        x.reshape(b, R, N, C).transpose(0, 1, 3, 2)).astype(np.float16)
    in_maps = []
    for core in range(NCORES):
        bi, hpi = divmod(core, 4)
        cols = slice(hpi * HP, (hpi + 1) * HP)
        in_maps.append({
            "xT": xT[bi],
            "wq": np.ascontiguousarray(Wq[:, cols]).astype(np.float16),
            "wk": np.ascontiguousarray(Wkv[:, cols]).astype(np.float16),
            "wv": np.ascontiguousarray(
                Wkv[:, 512 + hpi * HP: 512 + (hpi + 1) * HP]).astype(np.float16),
            "wo": np.ascontiguousarray(Wo[cols, :]).astype(np.float16),
        })
    return in_maps


def combine_outputs(ys, bo):
    """ys: list of 8 [R, N, E] f16 partials in core order; returns [B, n, dim]."""
    ys = [np.asarray(t, np.float32) for t in ys]
    y0 = ys[0] + ys[1] + ys[2] + ys[3]
    y1 = ys[4] + ys[5] + ys[6] + ys[7]
    yy = np.concatenate([y0, y1], axis=0).reshape(2 * R, N, E)
    return (yy + np.asarray(bo, np.float32)).astype(np.float32)


def kernel(x, Wq, Wkv, Wo, bo, tie_attn_dim):
    assert int(tie_attn_dim) == R, f"hardcoded for tie_attn_dim={R}"
    from concourse.bass_utils import run_bass_kernel_spmd

    nc = _get_program()
    in_maps = make_in_maps(x, Wq, Wkv, Wo)
    res = run_bass_kernel_spmd(nc, in_maps, list(range(NCORES)))
    ys = [np.asarray(res.results[c]["y"], np.float32) for c in range(NCORES)]
    return combine_outputs(ys, bo)


# revision 61
# speedup vs baseline: 1.3631x; 1.0022x over previous
"""Tied-row (MSA) attention on 8 Trainium2 NeuronCores.

Reference computation (B=128, n=512, dim=256, h=8, dh=64,
r=tie_attn_dim=64, b=B//r=2):
    q = x @ Wq ; k,v = split(x @ Wkv)
    dots[b,h,i,j] = sum_{r,d} q[b,r,h,i,d] k[b,r,h,j,d] * scale
    attn = softmax_j(dots)
    out[b,r,h,i,d] = sum_j attn[b,h,i,j] v[b,r,h,j,d]
    y = out @ Wo + bo

Sharding: 8 cores = b(2) x head-pairs(4).  Each core owns one batch
element and 2 of the 8 heads and produces the partial
    y_part = out[:, :, own 2 heads, :] @ Wo[own 128 rows, :]
in f16; the host sums the 4 partials per b in f32 and adds bo.

Per-core device kernel (shapes hardcoded):
  inputs : xT [64, 256, 512] f16   (x[b] transposed to [r, c, n])
           wq,wk,wv [256, 128] f16 (wq pre-scaled by dh^-.5 * r^-.5)
           wo [128, 256] f16
  output : y  [64, 512, 256] f16   (partial)

All reductions feed the PE with full K=128 contraction chunks (the
cost dimension is the moving-free size only, so half-height K=64
matmuls waste PE):
  - dots contracts (r, d) in chunks of 128 by pairing consecutive MSA
    rows on the partition axis: qk2 [(r%2)*64+d, qk, h, rr, n] f16, so
    dots is 2h x 4i x 32rr matmuls of F=512 (vs 64 r-steps of K=64).
  - out[(r%2)*64+d, i] per (h, rr) uses v2 [j, jc, h, (r%2)*64+d] as
    stationary and attnT[h] [j, jc, i] as moving: 2h x 32rr x 4jc
    matmuls of F=512 with all 128 output partitions live.
  - y per r needs out in [hd, i] layout: 64-partition interleave
    copies out_ps[h][p*64:...] -> out_sb[h*64:...] recover K=128.

Cost-model facts that shape the schedule: a matmul costs its moving-
free size only (K and partitions are free, LDWEIGHTS unmodeled); an
engine op costs its free size (partitions free) plus ~0.3us fixed;
all DMA shares one 360 GB/s device, min ~0.6us issue; the PE's
p-state drops ~2-4x after any idle gap, so PE stalls are poison.

The (r%2) interleaves therefore avoid 64-partition engine copies
where possible: q/k take one cheap full-partition f32->f16 staging
copy per row (DVE/ACT alternate), and the partition scatter runs as
four big f16 SBUF->SBUF SWDGE DMAs per 8-row block on the otherwise
idle Pool queue; dots waves are gated on those landings (~8.5us
behind).  The last block interleaves directly on the engines so the
final waves don't wait for a DMA chain.  GPSIMD cannot touch PSUM
(BIR verifier), so Pool gets only SBUF work: the interleave DMAs,
softmax scale, and the y writeback DMAs.

Phase 1 streams x once computing q/k per row; dots i-tiles 0,1
accumulate a block behind the staging (PSUM: 2x2 qk + 4 dots).
Wave-two dots tiles share the wave-one tag, so they rotate into the
same banks one exp-latency behind the wave-one softmax -- no pool-
scope boundary stall.  Phase 3 reloads x (v proj), then out/y in a
B/A/A/C software pipeline (PSUM: 2 v + 2 out + 2x2 y); v's head
interleave is a single multi-dim-AP copy (h is free on both sides),
out's needs the 64-partition copies (h moves to partitions).

Built with bacc.Bacc(): its compile() pass legalizes Tile's sync for
walrus (which caps sync waits per instruction); callers must
finalize() the program before running (see _get_program).
"""

import os
import sys

for _p in ("/opt/trn_rl_repo", "/root/.axon_site/_ro/trn_rl_repo"):
    if os.path.isdir(_p) and _p not in sys.path:
        sys.path.insert(0, _p)

import numpy as np
from collections import deque

R = 64          # tie dim (MSA rows per batch element)
RR = 32         # r-pairs
RB = 8          # rows per qk staging/interleave block
XB = 4          # rows per x DMA block
RBY = 2         # rows per y DMA block
N = 512         # sequence length
C = 256         # model dim
HP = 128        # head-pair width: 2 heads x 64
E = 256         # output dim
NCORES = 8

_CACHE = {}


def build_program(phases=(1, 2, 3)):
    import concourse.bacc as bacc
    from concourse import mybir
    from concourse.tile import TileContext
    from contextlib import ExitStack

    f32 = mybir.dt.float32
    f16 = mybir.dt.float16

    nc = bacc.Bacc()
    xT = nc.declare_dram_parameter("xT", [R, C, N], f16, isOutput=False)
    wq = nc.declare_dram_parameter("wq", [C, HP], f16, isOutput=False)
    wk = nc.declare_dram_parameter("wk", [C, HP], f16, isOutput=False)
    wv = nc.declare_dram_parameter("wv", [C, HP], f16, isOutput=False)
    wo = nc.declare_dram_parameter("wo", [HP, E], f16, isOutput=False)
    y = nc.declare_dram_parameter("y", [R, N, E], f16, isOutput=True)

    xT_blk = xT.rearrange("(rb r) (cc p) n -> rb p r cc n", r=XB, p=128)
    y_blk = y.rearrange("(rb r) (t p) e -> rb p r t e", r=RBY, p=128)

    def copy_eng(e, out, in_):
        if e % 2 == 0:
            nc.vector.tensor_copy(out, in_)
        else:
            nc.scalar.copy(out, in_)

    with TileContext(nc) as tc, ExitStack() as ctx:
        singles = ctx.enter_context(tc.tile_pool(name="singles", bufs=1))
        sm = ctx.enter_context(tc.tile_pool(name="sm", bufs=4))
        attnp = ctx.enter_context(tc.tile_pool(name="attnp", bufs=4))
        attntp = ctx.enter_context(tc.tile_pool(name="attntp", bufs=2))
        xpool = ctx.enter_context(tc.tile_pool(name="xpool", bufs=3))
        qknp = ctx.enter_context(tc.tile_pool(name="qknp", bufs=4))

        # weights: [256, X] -> sbuf [128, 2, X] (c-chunk on free axis)
        wq_sb = singles.tile([128, 2, HP], f16)
        wk_sb = singles.tile([128, 2, HP], f16)
        wv_sb = singles.tile([128, 2, HP], f16)
        wo_sb = singles.tile([128, E], f16)

        # attnT survives into phase 3: kernel-scoped pool
        attnT = [attntp.tile([128, 4, N], f16, tag="attnT", name=f"attnT_{h}")
                 for h in range(2)]

        def softmax(dots_hit, h, it):
            """dots PSUM tile -> normalized f16 attn SBUF tile.

            No max-subtraction: dots = q k^T with the 1/(sqrt(dh) sqrt(r))
            scale folded into Wq, so entries are ~N(0,1) and exp cannot
            overflow fp32/fp16."""
            ssum = sm.tile([128, 1], f32, tag="ssum", bufs=8)
            rinv = sm.tile([128, 1], f32, tag="rinv", bufs=8)
            attn = attnp.tile([128, N], f16, tag="attn", bufs=4,
                              name=f"attn_{h}_{it}")
            nc.scalar.activation(
                out=attn, in_=dots_hit,
                func=mybir.ActivationFunctionType.Exp,
                accum_out=ssum)
            nc.vector.reciprocal(rinv, ssum)
            nc.gpsimd.tensor_scalar_mul(attn, attn, rinv)
            return attn

        def transpose_attn(attn, h, it):
            # one f16 xbar DMA transpose, SBUF -> SBUF: out[j, jc, i] =
            # attn[i, jc*128 + j]; no PE/PSUM involvement
            nc.sync.dma_start_transpose(
                out=attnT[h][:, :, it * 128:(it + 1) * 128], in_=attn)

        # resident interleaved q/k, one tile: [(r%2)*64+d, rr, qk, h, n] f16
        with tc.tile_pool(name="resid", bufs=1) as resid:
            qk2 = resid.tile([128, RR, 2, 2, N], f16)

            def dots_mm(tile, h, ic, rr):
                nc.tensor.matmul(
                    tile,
                    lhsT=qk2[:, rr, 0, h, ic * 128:(ic + 1) * 128],
                    rhs=qk2[:, rr, 1, h, :],
                    start=(rr == 0), stop=(rr == RR - 1))

            # -------- Phase 1 + dots i-tiles 0,1 fused --------
            attn01 = {}
            qk_nats = {}
            with tc.tile_pool(name="ps1", space="PSUM", bufs=2) as ps1:
                dots01 = [[ps1.tile([128, N], f32, tag="dots", bufs=4,
                                    name=f"dotsA_{h}_{ic}")
                           for ic in range(2)] for h in range(2)]
                n_r = R if 1 in phases else 0
                next_rr = 0
                pending_ints = deque()

                def emit_dots_up_to(limit, budget):
                    nonlocal next_rr
                    while next_rr < min(limit, RR) and budget > 0:
                        for h in range(2):
                            for ic in (0, 1):
                                dots_mm(dots01[h][ic], h, ic, next_rr)
                        next_rr += 1
                        budget -= 1

                for r in range(n_r + 16):
                    if r == 0:
                        # split first x block so the first projection can
                        # start early, and load weights behind it
                        x_sb = xpool.tile([128, XB, 2, N], f16, tag="x",
                                          name="x1_0")
                        nc.sync.dma_start(out=x_sb[:, 0:2], in_=xT_blk[0, :, 0:2])
                        for cc in range(2):
                            nc.sync.dma_start(out=wq_sb[:, cc, :],
                                              in_=wq[cc * 128:(cc + 1) * 128, :])
                            nc.sync.dma_start(out=wk_sb[:, cc, :],
                                              in_=wk[cc * 128:(cc + 1) * 128, :])
                        nc.sync.dma_start(out=x_sb[:, 2:XB], in_=xT_blk[0, :, 2:XB])
                        for cc in range(2):
                            nc.sync.dma_start(out=wv_sb[:, cc, :],
                                              in_=wv[cc * 128:(cc + 1) * 128, :])
                        nc.sync.dma_start(out=wo_sb, in_=wo[:, :])
                    if r < n_r:
                        rb, ri = divmod(r, XB)
                        if ri == 0 and rb > 0:
                            x_sb = xpool.tile([128, XB, 2, N], f16, tag="x",
                                              name=f"x1_{rb}")
                            nc.sync.dma_start(out=x_sb, in_=xT_blk[rb])
                        qk_ps = ps1.tile([128, 2, N], f32, tag="qk")
                        for cc in range(2):
                            nc.tensor.matmul(qk_ps[:, 0, :], lhsT=wq_sb[:, cc, :],
                                             rhs=x_sb[:, ri, cc, :],
                                             start=(cc == 0), stop=(cc == 1))
                        for cc in range(2):
                            nc.tensor.matmul(qk_ps[:, 1, :], lhsT=wk_sb[:, cc, :],
                                             rhs=x_sb[:, ri, cc, :],
                                             start=(cc == 0), stop=(cc == 1))
                        # one cheap full-partition f32->f16 staging copy per
                        # row (engines alternate); once 8 rows are staged,
                        # the 64-partition (h -> r%2) interleave runs as 4
                        # big f16 SBUF->SBUF SWDGE DMAs from the Pool queue
                        # (its own DGE queue: no head-of-line blocking
                        # behind SP's x loads).
                        blk, rb8 = divmod(r, RB)
                        if blk == n_r // RB - 1:
                            # last block: direct 64-partition interleave on
                            # the engines (they idle at the phase tail), so
                            # the final dots waves don't sit behind the
                            # ~8.5us staging-DMA chain
                            p, rr = r % 2, r // 2
                            ds = slice(p * 64, (p + 1) * 64)
                            nc.vector.tensor_copy(qk2[ds, rr, :, 0, :],
                                                  qk_ps[0:64, :, :])
                            nc.scalar.copy(qk2[ds, rr, :, 1, :],
                                           qk_ps[64:128, :, :])
                        else:
                            if rb8 == 0:
                                qk_nats[blk] = qknp.tile(
                                    [128, 2, 4, 2, N], f16, tag="qkn",
                                    bufs=2, name=f"qkn_{blk}")
                            copy_eng(r, qk_nats[blk][:, rb8 % 2, rb8 // 2, :, :],
                                     qk_ps)
                            if rb8 == RB - 1:
                                rrs = slice(blk * 4, (blk + 1) * 4)
                                for h in range(2):
                                    for pp in range(2):
                                        nc.gpsimd.dma_start(
                                            out=qk2[pp * 64:(pp + 1) * 64, rrs, :, h, :],
                                            in_=qk_nats[blk][h * 64:(h + 1) * 64, pp, :, :, :])
                    if 2 in phases and 1 in phases and r >= 11:
                        # last block's rows land fast (engine copies), the
                        # rest are gated on their staging DMAs
                        limit = min(4 * ((r - 11) // RB + 1),
                                    28 + max(0, (r - 57) // 2))
                        emit_dots_up_to(limit, 1 if r < n_r else RR)
                # wave-one softmax consumes the dots PSUM; wave-two tiles
                # rotate into the same tag's banks as the exps retire, so
                # the PE restarts on i-tiles 2,3 one exp-latency later
                # instead of waiting for a pool-scope boundary
                for h in range(2 if (2 in phases and 1 in phases) else 0):
                    for ic in (0, 1):
                        attn01[(h, ic)] = softmax(dots01[h][ic], h, ic)

                dots23 = [[ps1.tile([128, N], f32, tag="dots", bufs=4,
                                    name=f"dotsB_{h}_{ic}")
                           for ic in range(2)] for h in range(2)]
                for rr in range(RR if (2 in phases and 1 in phases) else 0):
                    for h in range(2):
                        for ic in (2, 3):
                            dots_mm(dots23[h][ic - 2], h, ic, rr)
                # wave-one transposes overlap wave two's accumulation
                for (h, ic), attn in attn01.items():
                    transpose_attn(attn, h, ic)
                for h in range(2 if (2 in phases and 1 in phases) else 0):
                    for ic in (2, 3):
                        attn = softmax(dots23[h][ic - 2], h, ic)
                        transpose_attn(attn, h, ic)

        # ---------------- Phase 3: v, out, y (SW pipeline) ---------
        with tc.tile_pool(name="ps3", space="PSUM", bufs=2) as ps3, \
             tc.tile_pool(name="vpool", bufs=3) as vpool, \
             tc.tile_pool(name="outp", bufs=4) as outp, \
             tc.tile_pool(name="ypool", bufs=2) as ypool:
            n_t = RR if 3 in phases else 0
            v2s = {}
            out_ps_s = {}
            y_sbs = {}

            def stage_a(r):
                rb, ri = divmod(r, XB)
                if ri == 0:
                    x_tiles[rb] = xpool.tile([128, XB, 2, N], f16,
                                             tag="x", name=f"x3_{rb}")
                    nc.sync.dma_start(out=x_tiles[rb], in_=xT_blk[rb])
                x_sb = x_tiles[rb]
                p, rr = r % 2, r // 2
                v_ps = ps3.tile([128, 4, 2, 64], f32, tag="v", name=f"v_ps_{r}")
                for jt in range(4):
                    for cc in range(2):
                        nc.tensor.matmul(
                            v_ps[:, jt],
                            lhsT=x_sb[:, ri, cc, jt * 128:(jt + 1) * 128],
                            rhs=wv_sb[:, cc, :],
                            start=(cc == 0), stop=(cc == 1))
                if p == 0:
                    v2s[rr] = vpool.tile([128, 4, 2, 128], f16, tag="v2",
                                         name=f"v2_{rr}")
                # both heads in one multi-dim-AP copy: src free (jt, h, d),
                # dst free (jc, h, d) with the parity offset on d
                copy_eng(r, v2s[rr][:, :, :, p * 64:(p + 1) * 64], v_ps)

            def stage_b(rr):
                o = [ps3.tile([128, N], f32, tag="out", bufs=2,
                              name=f"out_ps_{rr}_{h}") for h in range(2)]
                v2 = v2s.pop(rr)
                for jc in range(4):
                    for h in range(2):
                        nc.tensor.matmul(
                            o[h],
                            lhsT=v2[:, jc, h, :],
                            rhs=attnT[h][:, jc, :],
                            start=(jc == 0), stop=(jc == 3))
                out_ps_s[rr] = o

            def stage_c(r):
                p, rr = r % 2, r // 2
                o = out_ps_s[rr]
                out_sb = outp.tile([128, N], f16, tag="outsb",
                                   name=f"out_sb_{r}")
                for h in range(2):
                    copy_eng(r + h,
                             out_sb[h * 64:(h + 1) * 64, :],
                             o[h][p * 64:(p + 1) * 64, :])
                if p == 1:
                    out_ps_s.pop(rr)
                y_ps = ps3.tile([128, 4, E], f32, tag="y", bufs=2,
                                name=f"y_ps_{r}")
                for ic in range(4):
                    nc.tensor.matmul(
                        y_ps[:, ic, :],
                        lhsT=out_sb[:, ic * 128:(ic + 1) * 128],
                        rhs=wo_sb,
                        start=True, stop=True)
                yb, ryi = divmod(r, RBY)
                if ryi == 0:
                    y_sbs[yb] = ypool.tile([128, RBY, 4, E], f16, tag="ysb",
                                           name=f"y_sb_{yb}")
                if r == R - 1:
                    # final row: split the copy across both engines and DMA
                    # the two rows separately so the program tail is short
                    nc.vector.tensor_copy(y_sbs[yb][:, ryi, 0:2, :],
                                          y_ps[:, 0:2, :])
                    nc.scalar.copy(y_sbs[yb][:, ryi, 2:4, :], y_ps[:, 2:4, :])
                    nc.gpsimd.dma_start(out=y_blk[yb][:, 0:1],
                                        in_=y_sbs[yb][:, 0:1])
                    nc.sync.dma_start(out=y_blk[yb][:, 1:2],
                                      in_=y_sbs.pop(yb)[:, 1:2])
                else:
                    copy_eng(r, y_sbs[yb][:, ryi, :, :], y_ps)
                    if ryi == RBY - 1:
                        nc.gpsimd.dma_start(out=y_blk[yb], in_=y_sbs.pop(yb))

            # B first, then the next pair's A as PE filler while the
            # out copies land, then C whose y matmuls need those copies
            x_tiles = {}
            for t in range(n_t + 2):
                if 0 <= t - 1 < n_t:
                    stage_b(t - 1)
                if t < n_t:
                    stage_a(2 * t)
                    stage_a(2 * t + 1)
                if 0 <= t - 1 < n_t:
                    stage_c(2 * (t - 1))
                    stage_c(2 * (t - 1) + 1)

    return nc


def _get_program():
    if "nc" not in _CACHE:
        nc = build_program()
        nc.finalize()
        _CACHE["nc"] = nc
    return _CACHE["nc"]


def make_in_maps(x, Wq, Wkv, Wo):
    """Host-side sharding: core = bi*4 + hpi."""
    scale = (64.0 ** -0.5) * (64.0 ** -0.5)
    x = np.asarray(x, np.float32)
    Wq = np.asarray(Wq, np.float32) * np.float32(scale)
    Wkv = np.asarray(Wkv, np.float32)
    Wo = np.asarray(Wo, np.float32)
    b = x.shape[0] // R
    xT = np.ascontiguousarray(
        x.reshape(b, R, N, C).transpose(0, 1, 3, 2)).astype(np.float16)
    in_maps = []
    for core in range(NCORES):
        bi, hpi = divmod(core, 4)
        cols = slice(hpi * HP, (hpi + 1) * HP)
        in_maps.append({
            "xT": xT[bi],
            "wq": np.ascontiguousarray(Wq[:, cols]).astype(np.float16),
            "wk": np.ascontiguousarray(Wkv[:, cols]).astype(np.float16),
            "wv": np.ascontiguousarray(
                Wkv[:, 512 + hpi * HP: 512 + (hpi + 1) * HP]).astype(np.float16),
            "wo": np.ascontiguousarray(Wo[cols, :]).astype(np.float16),
        })
    return in_maps


def combine_outputs(ys, bo):
    """ys: list of 8 [R, N, E] f16 partials in core order; returns [B, n, dim]."""
    ys = [np.asarray(t, np.float32) for t in ys]
    y0 = ys[0] + ys[1] + ys[2] + ys[3]
    y1 = ys[4] + ys[5] + ys[6] + ys[7]
    yy = np.concatenate([y0, y1], axis=0).reshape(2 * R, N, E)
    return (yy + np.asarray(bo, np.float32)).astype(np.float32)


def kernel(x, Wq, Wkv, Wo, bo, tie_attn_dim):
    assert int(tie_attn_dim) == R, f"hardcoded for tie_attn_dim={R}"
    from concourse.bass_utils import run_bass_kernel_spmd

    nc = _get_program()
    in_maps = make_in_maps(x, Wq, Wkv, Wo)
    res = run_bass_kernel_spmd(nc, in_maps, list(range(NCORES)))
    ys = [np.asarray(res.results[c]["y"], np.float32) for c in range(NCORES)]
    return combine_outputs(ys, bo)


# revision 78
# speedup vs baseline: 1.4156x; 1.0385x over previous
"""Tied-row (MSA) attention on 8 Trainium2 NeuronCores.

Reference computation (B=128, n=512, dim=256, h=8, dh=64,
r=tie_attn_dim=64, b=B//r=2):
    q = x @ Wq ; k,v = split(x @ Wkv)
    dots[b,h,i,j] = sum_{r,d} q[b,r,h,i,d] k[b,r,h,j,d] * scale
    attn = softmax_j(dots)
    out[b,r,h,i,d] = sum_j attn[b,h,i,j] v[b,r,h,j,d]
    y = out @ Wo + bo

Sharding: 8 cores = b(2) x head-pairs(4).  Each core owns one batch
element and 2 of the 8 heads and produces the partial
    y_part = out[:, :, own 2 heads, :] @ Wo[own 128 rows, :]
in f16; the host sums the 4 partials per b in f32 and adds bo.

Per-core device kernel (shapes hardcoded):
  inputs : xT [64, 256, 512] f16   (x[b] transposed to [r, c, n])
           wq,wk,wv [256, 128] f16 (wq pre-scaled by dh^-.5 * r^-.5)
           wo [128, 256] f16
  output : y  [64, 512, 256] f16   (partial)

All reductions feed the PE with full K=128 contraction chunks (the
cost dimension is the moving-free size only, so half-height K=64
matmuls waste PE):
  - dots contracts (r, d) in chunks of 128 by pairing consecutive MSA
    rows on the partition axis: qk2 [(r%2)*64+d, qk, h, rr, n] f16, so
    dots is 2h x 4i x 32rr matmuls of F=512 (vs 64 r-steps of K=64).
  - out[(r%2)*64+d, i] per (h, rr) uses v2 [j, jc, h, (r%2)*64+d] as
    stationary and attnT[h] [j, jc, i] as moving: 2h x 32rr x 4jc
    matmuls of F=512 with all 128 output partitions live.
  - y per r needs out in [hd, i] layout: 64-partition interleave
    copies out_ps[h][p*64:...] -> out_sb[h*64:...] recover K=128.

Cost-model facts that shape the schedule: a matmul costs its moving-
free size only (K and partitions are free, LDWEIGHTS unmodeled); an
engine op costs its free size (partitions free) plus ~0.3us fixed;
all DMA shares one 360 GB/s device, min ~0.6us issue; the PE's
p-state drops ~2-4x after any idle gap, so PE stalls are poison.

The (r%2) interleaves therefore avoid 64-partition engine copies
where possible: q/k take one cheap full-partition f32->f16 staging
copy per row (DVE/ACT alternate), and the partition scatter runs as
four big f16 SBUF->SBUF SWDGE DMAs per 8-row block on the otherwise
idle Pool queue; dots waves are gated on those landings (~8.5us
behind).  The last block interleaves directly on the engines so the
final waves don't wait for a DMA chain.  GPSIMD cannot touch PSUM
(BIR verifier), so Pool gets only SBUF work: the interleave DMAs,
softmax scale, and the y writeback DMAs.

Phase 1 streams x once computing q/k per row; dots i-tiles 0,1
accumulate a block behind the staging (PSUM: 2x2 qk + 4 dots).
Wave-two dots tiles share the wave-one tag, so they rotate into the
same banks one exp-latency behind the wave-one softmax -- no pool-
scope boundary stall.  Phase 3 reloads x (v proj), then out/y in a
B/A/A/C software pipeline (PSUM: 2 v + 2 out + 2x2 y); v's head
interleave is a single multi-dim-AP copy (h is free on both sides),
out's needs the 64-partition copies (h moves to partitions).

Built with bacc.Bacc(): its compile() pass legalizes Tile's sync for
walrus (which caps sync waits per instruction); callers must
finalize() the program before running (see _get_program).
"""

import os
import sys

for _p in ("/opt/trn_rl_repo", "/root/.axon_site/_ro/trn_rl_repo"):
    if os.path.isdir(_p) and _p not in sys.path:
        sys.path.insert(0, _p)

import numpy as np
from collections import deque

R = 64          # tie dim (MSA rows per batch element)
RR = 32         # r-pairs
RB = 8          # rows per qk staging/interleave block
XB = 4          # rows per x DMA block
RBY = 2         # rows per y DMA block
N = 512         # sequence length
C = 256         # model dim
HP = 128        # head-pair width: 2 heads x 64
E = 256         # output dim
NCORES = 8

_CACHE = {}


def build_program(phases=(1, 2, 3)):
    import concourse.bacc as bacc
    from concourse import mybir
    from concourse.tile import TileContext
    from contextlib import ExitStack

    f32 = mybir.dt.float32
    f16 = mybir.dt.float16

    nc = bacc.Bacc()
    xT = nc.declare_dram_parameter("xT", [R, C, N], f16, isOutput=False)
    wq = nc.declare_dram_parameter("wq", [C, HP], f16, isOutput=False)
    wk = nc.declare_dram_parameter("wk", [C, HP], f16, isOutput=False)
    wv = nc.declare_dram_parameter("wv", [C, HP], f16, isOutput=False)
    wo = nc.declare_dram_parameter("wo", [HP, E], f16, isOutput=False)
    y = nc.declare_dram_parameter("y", [R, N, E], f16, isOutput=True)

    xT_blk = xT.rearrange("(rb r) (cc p) n -> rb p r cc n", r=XB, p=128)
    y_blk = y.rearrange("(rb r) (t p) e -> rb p r t e", r=RBY, p=128)

    def copy_eng(e, out, in_):
        if e % 2 == 0:
            nc.vector.tensor_copy(out, in_)
        else:
            nc.scalar.copy(out, in_)

    with TileContext(nc) as tc, ExitStack() as ctx:
        singles = ctx.enter_context(tc.tile_pool(name="singles", bufs=1))
        sm = ctx.enter_context(tc.tile_pool(name="sm", bufs=4))
        attnp = ctx.enter_context(tc.tile_pool(name="attnp", bufs=4))
        attntp = ctx.enter_context(tc.tile_pool(name="attntp", bufs=2))
        xpool = ctx.enter_context(tc.tile_pool(name="xpool", bufs=3))
        qknp = ctx.enter_context(tc.tile_pool(name="qknp", bufs=4))

        # weights: [256, X] -> sbuf [128, 2, X] (c-chunk on free axis)
        wq_sb = singles.tile([128, 2, HP], f16)
        wk_sb = singles.tile([128, 2, HP], f16)
        wv_sb = singles.tile([128, 2, HP], f16)
        wo_sb = singles.tile([128, E], f16)

        # attnT survives into phase 3: kernel-scoped pool
        attnT = [attntp.tile([128, 4, N], f16, tag="attnT", name=f"attnT_{h}")
                 for h in range(2)]

        def softmax(dots_hit, h, it):
            """dots PSUM tile -> normalized f16 attn SBUF tile.

            No max-subtraction: dots = q k^T with the 1/(sqrt(dh) sqrt(r))
            scale folded into Wq, so entries are ~N(0,1) and exp cannot
            overflow fp32/fp16."""
            ssum = sm.tile([128, 1], f32, tag="ssum", bufs=8)
            rinv = sm.tile([128, 1], f32, tag="rinv", bufs=8)
            attn = attnp.tile([128, N], f16, tag="attn", bufs=6,
                              name=f"attn_{h}_{it}")
            nc.scalar.activation(
                out=attn, in_=dots_hit,
                func=mybir.ActivationFunctionType.Exp,
                accum_out=ssum)
            nc.vector.reciprocal(rinv, ssum)
            nc.gpsimd.tensor_scalar_mul(attn, attn, rinv)
            return attn

        def transpose_attn(attn, h, it):
            # one f16 xbar DMA transpose, SBUF -> SBUF: out[j, jc, i] =
            # attn[i, jc*128 + j]; no PE/PSUM involvement
            nc.sync.dma_start_transpose(
                out=attnT[h][:, :, it * 128:(it + 1) * 128], in_=attn)

        # resident interleaved q/k, one tile: [(r%2)*64+d, rr, qk, h, n] f16
        with tc.tile_pool(name="resid", bufs=1) as resid:
            qk2 = resid.tile([128, RR, 2, 2, N], f16)

            def dots_mm(tile, h, ic, rr):
                nc.tensor.matmul(
                    tile,
                    lhsT=qk2[:, rr, 0, h, ic * 128:(ic + 1) * 128],
                    rhs=qk2[:, rr, 1, h, :],
                    start=(rr == 0), stop=(rr == RR - 1))

            # -------- Phase 1 + dots i-tiles 0,1 fused --------
            attn01 = {}
            qk_nats = {}
            with tc.tile_pool(name="ps1", space="PSUM", bufs=2) as ps1:
                dots01 = [[ps1.tile([128, N], f32, tag="dots", bufs=4,
                                    name=f"dotsA_{h}_{ic}")
                           for ic in range(2)] for h in range(2)]
                n_r = R if 1 in phases else 0
                next_rr = 0
                pending_ints = deque()

                def emit_dots_up_to(limit, budget):
                    nonlocal next_rr
                    while next_rr < min(limit, RR) and budget > 0:
                        for h in range(2):
                            for ic in (0, 1):
                                dots_mm(dots01[h][ic], h, ic, next_rr)
                        next_rr += 1
                        budget -= 1

                for r in range(n_r + 16):
                    if r == 0:
                        # split first x block so the first projection can
                        # start early, and load weights behind it
                        x_sb = xpool.tile([128, XB, 2, N], f16, tag="x",
                                          name="x1_0")
                        nc.gpsimd.dma_start(out=x_sb[:, 0:2],
                                            in_=xT_blk[0, :, 0:2])
                        nc.sync.dma_start(
                            out=wq_sb, in_=wq.rearrange("(cc p) hp -> p cc hp", p=128))
                        nc.sync.dma_start(
                            out=wk_sb, in_=wk.rearrange("(cc p) hp -> p cc hp", p=128))
                        nc.sync.dma_start(out=x_sb[:, 2:XB], in_=xT_blk[0, :, 2:XB])
                        for cc in range(2):
                            nc.sync.dma_start(out=wv_sb[:, cc, :],
                                              in_=wv[cc * 128:(cc + 1) * 128, :])
                        nc.sync.dma_start(out=wo_sb, in_=wo[:, :])
                    if r < n_r:
                        rb, ri = divmod(r, XB)
                        if ri == 0 and rb > 0:
                            x_sb = xpool.tile([128, XB, 2, N], f16, tag="x",
                                              name=f"x1_{rb}")
                            nc.sync.dma_start(out=x_sb, in_=xT_blk[rb])
                        qk_ps = ps1.tile([128, 2, N], f32, tag="qk")
                        for cc in range(2):
                            nc.tensor.matmul(qk_ps[:, 0, :], lhsT=wq_sb[:, cc, :],
                                             rhs=x_sb[:, ri, cc, :],
                                             start=(cc == 0), stop=(cc == 1))
                        for cc in range(2):
                            nc.tensor.matmul(qk_ps[:, 1, :], lhsT=wk_sb[:, cc, :],
                                             rhs=x_sb[:, ri, cc, :],
                                             start=(cc == 0), stop=(cc == 1))
                        # one cheap full-partition f32->f16 staging copy per
                        # row (engines alternate); once 8 rows are staged,
                        # the 64-partition (h -> r%2) interleave runs as 4
                        # big f16 SBUF->SBUF SWDGE DMAs from the Pool queue
                        # (its own DGE queue: no head-of-line blocking
                        # behind SP's x loads).
                        blk, rb8 = divmod(r, RB)
                        if blk == n_r // RB - 1:
                            # last block: direct 64-partition interleave on
                            # the engines (they idle at the phase tail), so
                            # the final dots waves don't sit behind the
                            # ~8.5us staging-DMA chain
                            p, rr = r % 2, r // 2
                            ds = slice(p * 64, (p + 1) * 64)
                            nc.vector.tensor_copy(qk2[ds, rr, :, 0, :],
                                                  qk_ps[0:64, :, :])
                            nc.scalar.copy(qk2[ds, rr, :, 1, :],
                                           qk_ps[64:128, :, :])
                        else:
                            if rb8 == 0:
                                qk_nats[blk] = qknp.tile(
                                    [128, 2, 4, 2, N], f16, tag="qkn",
                                    bufs=2, name=f"qkn_{blk}")
                            copy_eng(r, qk_nats[blk][:, rb8 % 2, rb8 // 2, :, :],
                                     qk_ps)
                            if rb8 == RB - 1:
                                rrs = slice(blk * 4, (blk + 1) * 4)
                                for h in range(2):
                                    for pp in range(2):
                                        nc.gpsimd.dma_start(
                                            out=qk2[pp * 64:(pp + 1) * 64, rrs, :, h, :],
                                            in_=qk_nats[blk][h * 64:(h + 1) * 64, pp, :, :, :])
                    if 2 in phases and 1 in phases and r >= 11:
                        # last block's rows land fast (engine copies), the
                        # rest are gated on their staging DMAs
                        limit = min(4 * ((r - 11) // RB + 1),
                                    28 + max(0, (r - 57) // 2))
                        emit_dots_up_to(limit, 1 if r < n_r else RR)
                # wave-one softmax consumes the dots PSUM; wave-two tiles
                # rotate into the same tag's banks as the exps retire, so
                # the PE restarts on i-tiles 2,3 one exp-latency later
                # instead of waiting for a pool-scope boundary
                for h in range(2 if (2 in phases and 1 in phases) else 0):
                    for ic in (0, 1):
                        attn01[(h, ic)] = softmax(dots01[h][ic], h, ic)

                # wave-two runs one (h, ic) tile at a time so each
                # softmax overlaps the next tile's accumulation and the
                # last PSUM banks free one exp-latency after the last wave
                if 2 in phases and 1 in phases:
                    for (h, ic), attn in attn01.items():
                        transpose_attn(attn, h, ic)
                    for wi, (ic, h) in enumerate(
                            [(2, 0), (2, 1), (3, 0), (3, 1)]):
                        tile = ps1.tile([128, N], f32, tag="dots", bufs=4,
                                        name=f"dotsB_{h}_{ic}")
                        for rr in range(RR):
                            dots_mm(tile, h, ic, rr)
                        attn = softmax(tile, h, ic)
                        transpose_attn(attn, h, ic)

        # ---------------- Phase 3: v, out, y (SW pipeline) ---------
        with tc.tile_pool(name="ps3", space="PSUM", bufs=2) as ps3, \
             tc.tile_pool(name="vpool", bufs=6) as vpool, \
             tc.tile_pool(name="outp", bufs=8) as outp, \
             tc.tile_pool(name="ypool", bufs=4) as ypool:
            n_t = RR if 3 in phases else 0
            v2s = {}
            out_ps_s = {}
            y_sbs = {}

            def stage_a(r):
                rb, ri = divmod(r, XB)
                if ri == 0:
                    x_tiles[rb] = xpool.tile([128, XB, 2, N], f16,
                                             tag="x", name=f"x3_{rb}")
                    nc.sync.dma_start(out=x_tiles[rb], in_=xT_blk[rb])
                x_sb = x_tiles[rb]
                p, rr = r % 2, r // 2
                v_ps = ps3.tile([128, 4, 2, 64], f32, tag="v", name=f"v_ps_{r}")
                for jt in range(4):
                    for cc in range(2):
                        nc.tensor.matmul(
                            v_ps[:, jt],
                            lhsT=x_sb[:, ri, cc, jt * 128:(jt + 1) * 128],
                            rhs=wv_sb[:, cc, :],
                            start=(cc == 0), stop=(cc == 1))
                if p == 0:
                    v2s[rr] = vpool.tile([128, 4, 2, 128], f16, tag="v2",
                                         name=f"v2_{rr}")
                # both heads in one multi-dim-AP copy: src free (jt, h, d),
                # dst free (jc, h, d) with the parity offset on d
                copy_eng(r, v2s[rr][:, :, :, p * 64:(p + 1) * 64], v_ps)

            def stage_b(rr):
                o = [ps3.tile([128, N], f32, tag="out", bufs=2,
                              name=f"out_ps_{rr}_{h}") for h in range(2)]
                v2 = v2s.pop(rr)
                for jc in range(4):
                    for h in range(2):
                        nc.tensor.matmul(
                            o[h],
                            lhsT=v2[:, jc, h, :],
                            rhs=attnT[h][:, jc, :],
                            start=(jc == 0), stop=(jc == 3))
                out_ps_s[rr] = o

            def stage_c(r):
                p, rr = r % 2, r // 2
                o = out_ps_s[rr]
                out_sb = outp.tile([128, N], f16, tag="outsb",
                                   name=f"out_sb_{r}")
                for h in range(2):
                    copy_eng(r + h,
                             out_sb[h * 64:(h + 1) * 64, :],
                             o[h][p * 64:(p + 1) * 64, :])
                if p == 1:
                    out_ps_s.pop(rr)
                y_ps = ps3.tile([128, 4, E], f32, tag="y", bufs=2,
                                name=f"y_ps_{r}")
                for ic in range(4):
                    nc.tensor.matmul(
                        y_ps[:, ic, :],
                        lhsT=out_sb[:, ic * 128:(ic + 1) * 128],
                        rhs=wo_sb,
                        start=True, stop=True)
                yb, ryi = divmod(r, RBY)
                if ryi == 0:
                    y_sbs[yb] = ypool.tile([128, RBY, 4, E], f16, tag="ysb",
                                           name=f"y_sb_{yb}")
                if r == R - 1:
                    # final row: split the copy across both engines and DMA
                    # the two rows separately so the program tail is short
                    nc.vector.tensor_copy(y_sbs[yb][:, ryi, 0:2, :],
                                          y_ps[:, 0:2, :])
                    nc.scalar.copy(y_sbs[yb][:, ryi, 2:4, :], y_ps[:, 2:4, :])
                    nc.gpsimd.dma_start(out=y_blk[yb][:, 0:1],
                                        in_=y_sbs[yb][:, 0:1])
                    nc.sync.dma_start(out=y_blk[yb][:, 1:2],
                                      in_=y_sbs.pop(yb)[:, 1:2])
                else:
                    copy_eng(r, y_sbs[yb][:, ryi, :, :], y_ps)
                    if ryi == RBY - 1:
                        nc.gpsimd.dma_start(out=y_blk[yb], in_=y_sbs.pop(yb))

            # B first, then the next pair's A as PE filler while the
            # out copies land, then C whose y matmuls need those copies
            x_tiles = {}
            for t in range(n_t + 2):
                if 0 <= t - 1 < n_t:
                    stage_b(t - 1)
                if t < n_t:
                    stage_a(2 * t)
                    stage_a(2 * t + 1)
                if 0 <= t - 1 < n_t:
                    stage_c(2 * (t - 1))
                    stage_c(2 * (t - 1) + 1)

    return nc


def _get_program():
    if "nc" not in _CACHE:
        nc = build_program()
        nc.finalize()
        _CACHE["nc"] = nc
    return _CACHE["nc"]


def make_in_maps(x, Wq, Wkv, Wo):
    """Host-side sharding: core = bi*4 + hpi."""
    scale = (64.0 ** -0.5) * (64.0 ** -0.5)
    x = np.asarray(x, np.float32)
    Wq = np.asarray(Wq, np.float32) * np.float32(scale)
    Wkv = np.asarray(Wkv, np.float32)
    Wo = np.asarray(Wo, np.float32)
    b = x.shape[0] // R
    xT = np.ascontiguousarray(
        x.reshape(b, R, N, C).transpose(0, 1, 3, 2)).astype(np.float16)
    in_maps = []
    for core in range(NCORES):
        bi, hpi = divmod(core, 4)
        cols = slice(hpi * HP, (hpi + 1) * HP)
        in_maps.append({
            "xT": xT[bi],
            "wq": np.ascontiguousarray(Wq[:, cols]).astype(np.float16),
            "wk": np.ascontiguousarray(Wkv[:, cols]).astype(np.float16),
            "wv": np.ascontiguousarray(
                Wkv[:, 512 + hpi * HP: 512 + (hpi + 1) * HP]).astype(np.float16),
            "wo": np.ascontiguousarray(Wo[cols, :]).astype(np.float16),
        })
    return in_maps


def combine_outputs(ys, bo):
    """ys: list of 8 [R, N, E] f16 partials in core order; returns [B, n, dim]."""
    ys = [np.asarray(t, np.float32) for t in ys]
    y0 = ys[0] + ys[1] + ys[2] + ys[3]
    y1 = ys[4] + ys[5] + ys[6] + ys[7]
    yy = np.concatenate([y0, y1], axis=0).reshape(2 * R, N, E)
    return (yy + np.asarray(bo, np.float32)).astype(np.float32)


def kernel(x, Wq, Wkv, Wo, bo, tie_attn_dim):
    assert int(tie_attn_dim) == R, f"hardcoded for tie_attn_dim={R}"
    from concourse.bass_utils import run_bass_kernel_spmd

    nc = _get_program()
    in_maps = make_in_maps(x, Wq, Wkv, Wo)
    res = run_bass_kernel_spmd(nc, in_maps, list(range(NCORES)))
    ys = [np.asarray(res.results[c]["y"], np.float32) for c in range(NCORES)]
    return combine_outputs(ys, bo)


# revision 80
# speedup vs baseline: 1.4253x; 1.0069x over previous
"""Tied-row (MSA) attention on 8 Trainium2 NeuronCores.

Reference computation (B=128, n=512, dim=256, h=8, dh=64,
r=tie_attn_dim=64, b=B//r=2):
    q = x @ Wq ; k,v = split(x @ Wkv)
    dots[b,h,i,j] = sum_{r,d} q[b,r,h,i,d] k[b,r,h,j,d] * scale
    attn = softmax_j(dots)
    out[b,r,h,i,d] = sum_j attn[b,h,i,j] v[b,r,h,j,d]
    y = out @ Wo + bo

Sharding: 8 cores = b(2) x head-pairs(4).  Each core owns one batch
element and 2 of the 8 heads and produces the partial
    y_part = out[:, :, own 2 heads, :] @ Wo[own 128 rows, :]
in f16; the host sums the 4 partials per b in f32 and adds bo.

Per-core device kernel (shapes hardcoded):
  inputs : xT [64, 256, 512] f16   (x[b] transposed to [r, c, n])
           wq,wk,wv [256, 128] f16 (wq pre-scaled by dh^-.5 * r^-.5)
           wo [128, 256] f16
  output : y  [64, 512, 256] f16   (partial)

All reductions feed the PE with full K=128 contraction chunks (the
cost dimension is the moving-free size only, so half-height K=64
matmuls waste PE):
  - dots contracts (r, d) in chunks of 128 by pairing consecutive MSA
    rows on the partition axis: qk2 [(r%2)*64+d, qk, h, rr, n] f16, so
    dots is 2h x 4i x 32rr matmuls of F=512 (vs 64 r-steps of K=64).
  - out[(r%2)*64+d, i] per (h, rr) uses v2 [j, jc, h, (r%2)*64+d] as
    stationary and attnT[h] [j, jc, i] as moving: 2h x 32rr x 4jc
    matmuls of F=512 with all 128 output partitions live.
  - y per r needs out in [hd, i] layout: 64-partition interleave
    copies out_ps[h][p*64:...] -> out_sb[h*64:...] recover K=128.

Cost-model facts that shape the schedule: a matmul costs its moving-
free size only (K and partitions are free, LDWEIGHTS unmodeled); an
engine op costs its free size (partitions free) plus ~0.3us fixed;
all DMA shares one 360 GB/s device, min ~0.6us issue; the PE's
p-state drops ~2-4x after any idle gap, so PE stalls are poison.

The (r%2) interleaves therefore avoid 64-partition engine copies
where possible: q/k take one cheap full-partition f32->f16 staging
copy per row (DVE/ACT alternate), and the partition scatter runs as
four big f16 SBUF->SBUF SWDGE DMAs per 8-row block on the otherwise
idle Pool queue; dots waves are gated on those landings (~8.5us
behind).  The last block interleaves directly on the engines so the
final waves don't wait for a DMA chain.  GPSIMD cannot touch PSUM
(BIR verifier), so Pool gets only SBUF work: the interleave DMAs,
softmax scale, and the y writeback DMAs.

Phase 1 streams x once computing q/k per row; dots i-tiles 0,1
accumulate a block behind the staging (PSUM: 2x2 qk + 4 dots).
Wave-two dots tiles share the wave-one tag, so they rotate into the
same banks one exp-latency behind the wave-one softmax -- no pool-
scope boundary stall.  Phase 3 reloads x (v proj), then out/y in a
B/A/A/C software pipeline (PSUM: 2 v + 2 out + 2x2 y); v's head
interleave is a single multi-dim-AP copy (h is free on both sides),
out's needs the 64-partition copies (h moves to partitions).

Built with bacc.Bacc(): its compile() pass legalizes Tile's sync for
walrus (which caps sync waits per instruction); callers must
finalize() the program before running (see _get_program).
"""

import os
import sys

for _p in ("/opt/trn_rl_repo", "/root/.axon_site/_ro/trn_rl_repo"):
    if os.path.isdir(_p) and _p not in sys.path:
        sys.path.insert(0, _p)

import numpy as np
from collections import deque

R = 64          # tie dim (MSA rows per batch element)
RR = 32         # r-pairs
RB = 8          # rows per qk staging/interleave block
XB = 4          # rows per x DMA block
RBY = 2         # rows per y DMA block
N = 512         # sequence length
C = 256         # model dim
HP = 128        # head-pair width: 2 heads x 64
E = 256         # output dim
NCORES = 8

_CACHE = {}


def build_program(phases=(1, 2, 3)):
    import concourse.bacc as bacc
    from concourse import mybir
    from concourse.tile import TileContext
    from contextlib import ExitStack

    f32 = mybir.dt.float32
    f16 = mybir.dt.float16

    nc = bacc.Bacc()
    xT = nc.declare_dram_parameter("xT", [R, C, N], f16, isOutput=False)
    wq = nc.declare_dram_parameter("wq", [C, HP], f16, isOutput=False)
    wk = nc.declare_dram_parameter("wk", [C, HP], f16, isOutput=False)
    wv = nc.declare_dram_parameter("wv", [C, HP], f16, isOutput=False)
    wo = nc.declare_dram_parameter("wo", [HP, E], f16, isOutput=False)
    y = nc.declare_dram_parameter("y", [R, N, E], f16, isOutput=True)

    xT_blk = xT.rearrange("(rb r) (cc p) n -> rb p r cc n", r=XB, p=128)
    y_blk = y.rearrange("(rb r) (t p) e -> rb p r t e", r=RBY, p=128)

    def copy_eng(e, out, in_):
        if e % 2 == 0:
            nc.vector.tensor_copy(out, in_)
        else:
            nc.scalar.copy(out, in_)

    with TileContext(nc) as tc, ExitStack() as ctx:
        singles = ctx.enter_context(tc.tile_pool(name="singles", bufs=1))
        sm = ctx.enter_context(tc.tile_pool(name="sm", bufs=4))
        attnp = ctx.enter_context(tc.tile_pool(name="attnp", bufs=4))
        attntp = ctx.enter_context(tc.tile_pool(name="attntp", bufs=2))
        xpool = ctx.enter_context(tc.tile_pool(name="xpool", bufs=3))
        qknp = ctx.enter_context(tc.tile_pool(name="qknp", bufs=4))

        # weights: [256, X] -> sbuf [128, 2, X] (c-chunk on free axis)
        wq_sb = singles.tile([128, 2, HP], f16)
        wk_sb = singles.tile([128, 2, HP], f16)
        wv_sb = singles.tile([128, 2, HP], f16)
        wo_sb = singles.tile([128, E], f16)

        # attnT survives into phase 3: kernel-scoped pool
        attnT = [attntp.tile([128, 4, N], f16, tag="attnT", name=f"attnT_{h}")
                 for h in range(2)]

        def softmax(dots_hit, h, it):
            """dots PSUM tile -> normalized f16 attn SBUF tile.

            No max-subtraction: dots = q k^T with the 1/(sqrt(dh) sqrt(r))
            scale folded into Wq, so entries are ~N(0,1) and exp cannot
            overflow fp32/fp16."""
            ssum = sm.tile([128, 1], f32, tag="ssum", bufs=8)
            rinv = sm.tile([128, 1], f32, tag="rinv", bufs=8)
            attn = attnp.tile([128, N], f16, tag="attn", bufs=6,
                              name=f"attn_{h}_{it}")
            nc.scalar.activation(
                out=attn, in_=dots_hit,
                func=mybir.ActivationFunctionType.Exp,
                accum_out=ssum)
            nc.vector.reciprocal(rinv, ssum)
            nc.gpsimd.tensor_scalar_mul(attn, attn, rinv)
            return attn

        def transpose_attn(attn, h, it):
            # one f16 xbar DMA transpose, SBUF -> SBUF: out[j, jc, i] =
            # attn[i, jc*128 + j]; no PE/PSUM involvement
            nc.sync.dma_start_transpose(
                out=attnT[h][:, :, it * 128:(it + 1) * 128], in_=attn)

        # resident interleaved q/k, one tile: [(r%2)*64+d, rr, qk, h, n] f16
        with tc.tile_pool(name="resid", bufs=1) as resid:
            qk2 = resid.tile([128, RR, 2, 2, N], f16)

            def dots_mm(tile, h, ic, rr):
                nc.tensor.matmul(
                    tile,
                    lhsT=qk2[:, rr, 0, h, ic * 128:(ic + 1) * 128],
                    rhs=qk2[:, rr, 1, h, :],
                    start=(rr == 0), stop=(rr == RR - 1))

            # -------- Phase 1 + dots i-tiles 0,1 fused --------
            attn01 = {}
            qk_nats = {}
            with tc.tile_pool(name="ps1", space="PSUM", bufs=2) as ps1:
                dots01 = [[ps1.tile([128, N], f32, tag="dots", bufs=4,
                                    name=f"dotsA_{h}_{ic}")
                           for ic in range(2)] for h in range(2)]
                n_r = R if 1 in phases else 0
                next_rr = 0
                pending_ints = deque()

                def emit_dots_up_to(limit, budget):
                    nonlocal next_rr
                    while next_rr < min(limit, RR) and budget > 0:
                        for h in range(2):
                            for ic in (0, 1):
                                dots_mm(dots01[h][ic], h, ic, next_rr)
                        next_rr += 1
                        budget -= 1

                for r in range(n_r + 16):
                    if r == 0:
                        # split first x block so the first projection can
                        # start early, and load weights behind it
                        x_sb = xpool.tile([128, XB, 2, N], f16, tag="x",
                                          name="x1_0")
                        nc.gpsimd.dma_start(out=x_sb[:, 0:2],
                                            in_=xT_blk[0, :, 0:2])
                        nc.sync.dma_start(
                            out=wq_sb, in_=wq.rearrange("(cc p) hp -> p cc hp", p=128))
                        nc.sync.dma_start(
                            out=wk_sb, in_=wk.rearrange("(cc p) hp -> p cc hp", p=128))
                        nc.sync.dma_start(out=x_sb[:, 2:XB], in_=xT_blk[0, :, 2:XB])
                        for cc in range(2):
                            nc.sync.dma_start(out=wv_sb[:, cc, :],
                                              in_=wv[cc * 128:(cc + 1) * 128, :])
                        nc.sync.dma_start(out=wo_sb, in_=wo[:, :])
                    if r < n_r:
                        rb, ri = divmod(r, XB)
                        if ri == 0 and rb > 0:
                            x_sb = xpool.tile([128, XB, 2, N], f16, tag="x",
                                              name=f"x1_{rb}")
                            nc.sync.dma_start(out=x_sb, in_=xT_blk[rb])
                        qk_ps = ps1.tile([128, 2, N], f32, tag="qk")
                        for cc in range(2):
                            nc.tensor.matmul(qk_ps[:, 0, :], lhsT=wq_sb[:, cc, :],
                                             rhs=x_sb[:, ri, cc, :],
                                             start=(cc == 0), stop=(cc == 1))
                        for cc in range(2):
                            nc.tensor.matmul(qk_ps[:, 1, :], lhsT=wk_sb[:, cc, :],
                                             rhs=x_sb[:, ri, cc, :],
                                             start=(cc == 0), stop=(cc == 1))
                        # one cheap full-partition f32->f16 staging copy per
                        # row (engines alternate); once 8 rows are staged,
                        # the 64-partition (h -> r%2) interleave runs as 4
                        # big f16 SBUF->SBUF SWDGE DMAs from the Pool queue
                        # (its own DGE queue: no head-of-line blocking
                        # behind SP's x loads).
                        blk, rb8 = divmod(r, RB)
                        if blk == n_r // RB - 1:
                            # last block: direct 64-partition interleave on
                            # the engines (they idle at the phase tail), so
                            # the final dots waves don't sit behind the
                            # ~8.5us staging-DMA chain
                            p, rr = r % 2, r // 2
                            ds = slice(p * 64, (p + 1) * 64)
                            nc.vector.tensor_copy(qk2[ds, rr, :, 0, :],
                                                  qk_ps[0:64, :, :])
                            nc.scalar.copy(qk2[ds, rr, :, 1, :],
                                           qk_ps[64:128, :, :])
                        else:
                            if rb8 == 0:
                                qk_nats[blk] = qknp.tile(
                                    [128, 2, 4, 2, N], f16, tag="qkn",
                                    bufs=2, name=f"qkn_{blk}")
                            copy_eng(r, qk_nats[blk][:, rb8 % 2, rb8 // 2, :, :],
                                     qk_ps)
                            if rb8 == RB - 1:
                                rrs = slice(blk * 4, (blk + 1) * 4)
                                for h in range(2):
                                    for pp in range(2):
                                        nc.gpsimd.dma_start(
                                            out=qk2[pp * 64:(pp + 1) * 64, rrs, :, h, :],
                                            in_=qk_nats[blk][h * 64:(h + 1) * 64, pp, :, :, :])
                    if 2 in phases and 1 in phases and r >= 11:
                        # last block's rows land fast (engine copies), the
                        # rest are gated on their staging DMAs
                        limit = min(4 * ((r - 11) // RB + 1),
                                    28 + max(0, (r - 57) // 2))
                        emit_dots_up_to(limit, 1 if r < n_r else RR)
                # wave-one softmax consumes the dots PSUM; wave-two tiles
                # rotate into the same tag's banks as the exps retire, so
                # the PE restarts on i-tiles 2,3 one exp-latency later
                # instead of waiting for a pool-scope boundary
                for h in range(2 if (2 in phases and 1 in phases) else 0):
                    for ic in (0, 1):
                        attn01[(h, ic)] = softmax(dots01[h][ic], h, ic)

                # wave-two runs one (h, ic) tile at a time so each
                # softmax overlaps the next tile's accumulation and the
                # last PSUM banks free one exp-latency after the last wave
                if 2 in phases and 1 in phases:
                    for (h, ic), attn in attn01.items():
                        transpose_attn(attn, h, ic)
                    for wi, (ic, h) in enumerate(
                            [(2, 0), (2, 1), (3, 0), (3, 1)]):
                        tile = ps1.tile([128, N], f32, tag="dots", bufs=4,
                                        name=f"dotsB_{h}_{ic}")
                        for rr in range(RR):
                            dots_mm(tile, h, ic, rr)
                        attn = softmax(tile, h, ic)
                        transpose_attn(attn, h, ic)

        # ---------------- Phase 3: v, out, y (SW pipeline) ---------
        with tc.tile_pool(name="ps3", space="PSUM", bufs=2) as ps3, \
             tc.tile_pool(name="vpool", bufs=6) as vpool, \
             tc.tile_pool(name="outp", bufs=8) as outp, \
             tc.tile_pool(name="ypool", bufs=4) as ypool:
            n_t = RR if 3 in phases else 0
            v2s = {}
            out_ps_s = {}
            y_sbs = {}

            def stage_a(r):
                rb, ri = divmod(r, XB)
                if ri == 0:
                    x_tiles[rb] = xpool.tile([128, XB, 2, N], f16,
                                             tag="x", name=f"x3_{rb}")
                    nc.sync.dma_start(out=x_tiles[rb], in_=xT_blk[rb])
                x_sb = x_tiles[rb]
                p, rr = r % 2, r // 2
                v_ps = ps3.tile([128, 4, 2, 64], f32, tag="v", name=f"v_ps_{r}")
                for jt in range(4):
                    for cc in range(2):
                        nc.tensor.matmul(
                            v_ps[:, jt],
                            lhsT=x_sb[:, ri, cc, jt * 128:(jt + 1) * 128],
                            rhs=wv_sb[:, cc, :],
                            start=(cc == 0), stop=(cc == 1))
                if p == 0:
                    v2s[rr] = vpool.tile([128, 4, 2, 128], f16, tag="v2",
                                         name=f"v2_{rr}")
                # both heads in one multi-dim-AP copy: src free (jt, h, d),
                # dst free (jc, h, d) with the parity offset on d
                copy_eng(r, v2s[rr][:, :, :, p * 64:(p + 1) * 64], v_ps)

            def stage_b(rr):
                o = [ps3.tile([128, N], f32, tag="out", bufs=2,
                              name=f"out_ps_{rr}_{h}") for h in range(2)]
                v2 = v2s.pop(rr)
                for jc in range(4):
                    for h in range(2):
                        nc.tensor.matmul(
                            o[h],
                            lhsT=v2[:, jc, h, :],
                            rhs=attnT[h][:, jc, :],
                            start=(jc == 0), stop=(jc == 3))
                out_ps_s[rr] = o

            def stage_c(r):
                p, rr = r % 2, r // 2
                o = out_ps_s[rr]
                out_sb = outp.tile([128, N], f16, tag="outsb",
                                   name=f"out_sb_{r}")
                for h in range(2):
                    copy_eng(r + h,
                             out_sb[h * 64:(h + 1) * 64, :],
                             o[h][p * 64:(p + 1) * 64, :])
                if p == 1:
                    out_ps_s.pop(rr)
                y_ps = ps3.tile([128, 4, E], f32, tag="y", bufs=2,
                                name=f"y_ps_{r}")
                for ic in range(4):
                    nc.tensor.matmul(
                        y_ps[:, ic, :],
                        lhsT=out_sb[:, ic * 128:(ic + 1) * 128],
                        rhs=wo_sb,
                        start=True, stop=True)
                yb, ryi = divmod(r, RBY)
                if ryi == 0:
                    y_sbs[yb] = ypool.tile([128, RBY, 4, E], f16, tag="ysb",
                                           name=f"y_sb_{yb}")
                if r == R - 1:
                    # final row: split the copy across both engines and DMA
                    # the two rows separately so the program tail is short
                    nc.vector.tensor_copy(y_sbs[yb][:, ryi, 0:2, :],
                                          y_ps[:, 0:2, :])
                    nc.scalar.copy(y_sbs[yb][:, ryi, 2:4, :], y_ps[:, 2:4, :])
                    nc.gpsimd.dma_start(out=y_blk[yb][:, 0:1],
                                        in_=y_sbs[yb][:, 0:1])
                    nc.sync.dma_start(out=y_blk[yb][:, 1:2],
                                      in_=y_sbs.pop(yb)[:, 1:2])
                else:
                    copy_eng(r, y_sbs[yb][:, ryi, :, :], y_ps)
                    if ryi == RBY - 1:
                        nc.gpsimd.dma_start(out=y_blk[yb], in_=y_sbs.pop(yb))

            # B first, then the next pair's A as PE filler while the
            # out copies land, then C whose y matmuls need those copies
            x_tiles = {}
            for t in range(n_t + 2):
                if 0 <= t - 1 < n_t:
                    stage_b(t - 1)
                if t < n_t:
                    stage_a(2 * t)
                    stage_a(2 * t + 1)
                if 0 <= t - 1 < n_t:
                    stage_c(2 * (t - 1))
                    stage_c(2 * (t - 1) + 1)

    return nc


def _get_program():
    if "nc" not in _CACHE:
        nc = build_program()
        nc.finalize()
        _CACHE["nc"] = nc
    return _CACHE["nc"]


def make_in_maps(x, Wq, Wkv, Wo):
    """Host-side sharding: core = bi*4 + hpi."""
    scale = (64.0 ** -0.5) * (64.0 ** -0.5)
    x = np.asarray(x, np.float32)
    Wq = np.asarray(Wq, np.float32) * np.float32(scale)
    Wkv = np.asarray(Wkv, np.float32)
    Wo = np.asarray(Wo, np.float32)
    b = x.shape[0] // R
    xT = np.ascontiguousarray(
        x.reshape(b, R, N, C).transpose(0, 1, 3, 2)).astype(np.float16)
    in_maps = []
    for core in range(NCORES):
        bi, hpi = divmod(core, 4)
        cols = slice(hpi * HP, (hpi + 1) * HP)
        in_maps.append({
            "xT": xT[bi],
            "wq": np.ascontiguousarray(Wq[:, cols]).astype(np.float16),
            "wk": np.ascontiguousarray(Wkv[:, cols]).astype(np.float16),
            "wv": np.ascontiguousarray(
                Wkv[:, 512 + hpi * HP: 512 + (hpi + 1) * HP]).astype(np.float16),
            "wo": np.ascontiguousarray(Wo[cols, :]).astype(np.float16),
        })
    return in_maps


def combine_outputs(ys, bo):
    """ys: list of 8 [R, N, E] f16 partials in core order; returns [B, n, dim]."""
    ys = [np.asarray(t, np.float32) for t in ys]
    y0 = ys[0] + ys[1] + ys[2] + ys[3]
    y1 = ys[4] + ys[5] + ys[6] + ys[7]
    yy = np.concatenate([y0, y1], axis=0).reshape(2 * R, N, E)
    return (yy + np.asarray(bo, np.float32)).astype(np.float32)


def kernel(x, Wq, Wkv, Wo, bo, tie_attn_dim):
    assert int(tie_attn_dim) == R, f"hardcoded for tie_attn_dim={R}"
    from concourse.bass_utils import run_bass_kernel_spmd

    nc = _get_program()
    in_maps = make_in_maps(x, Wq, Wkv, Wo)
    res = run_bass_kernel_spmd(nc, in_maps, list(range(NCORES)))
    ys = [np.asarray(res.results[c]["y"], np.float32) for c in range(NCORES)]
    return combine_outputs(ys, bo)
